# revision 43
# baseline (speedup 1.0000x reference)
"""Multi-head cross-attention Trainium2 kernel.

Full-input contract: kernel(**inputs) takes the complete tensors and returns
the complete output. Internally shards over 8 NeuronCores as
(batch x head-group): core c handles batch c//4 and heads [4*(c%4), 4*(c%4)+4).
Each core computes its partial output  ctx_g @ Wo_g  for its batch; the host
sums the 4 head-group partials per batch and adds bo.

Masked keys (key_mask == 0) contribute exactly zero probability, so the host
compacts key/value to the unmasked rows (padded up to a multiple of 128 with
-1e9 score bias), which shrinks the K/V projections and the whole attention
core proportionally. The device program is built for the padded key count and
cached per count.

v2 design (vs the fp32r baseline; 185.8us -> 132.5us per the cost-model
timeline that the harness measures):
  - query/key/value and the QKV projection weights ship as bf16: same PE
    cost (1 cycle/row either way; fp8 was tested numerically and fails the
    2e-2 gate since score errors pass straight through softmax) but half
    the DMA bytes. All input DMAs ride the SP queue in exact need-order;
    the DMA device is a single serialized resource in the cost model, so
    queue order IS arrival order and the startup prefix paces the first
    two sk-blocks.
  - V is projected directly into [sk, dh] layout (lhsT = value^T chunk,
    rhs = Wv), which feeds the ctx matmul without the per-tile PE
    transposes the baseline needed.
  - K-projection only computes the t=0 half (heads 0/1) during the
    streaming phase; the t=1 half is injected into heads 1-2's sweeps,
    whose pace is otherwise ACT(exp)-bound, from a resident bf16 copy of
    key^T (kx_sb).
  - scores->exp->ctx is software-pipelined with ctx lagging 2 sk-tiles:
    exp(k+1) only waits on its own scores, reaching the pure exp-rate
    (1038ns/tile) instead of chaining sem+ctx+scores between exps.
  - The psum accumulator is spilled to SBUF at head end (frees the single
    [65,SQ] accumulator bank-pair immediately); softmax normalization
    (reciprocal + ones-broadcast matmul + multiply into ctxT) runs off the
    critical path inside the next head's sweep. The last head normalizes
    straight from psum in 256-col chunks with the bc-copy on ACT so the
    output projection unblocks early.
  - A chain of tiny warm-up matmuls runs during the initial DMA wait so
    the PE p-state ramp (2-3.7x slower matmuls after idle) is paid on junk
    work instead of the projections.
  - Output tiles are written bf16 (host sums partials in fp32); psum->sbuf
    copies split DVE(lo)/ACT(hi) per tile, one merged DMA per tile.

Per-core device pipeline:
  qT = (Wq_g^T @ query_b^T)          [256, 1024] fp32r
  per sk block: kT(t=0) block, v block (direct [sk,256] layout + bias into
  vext with a ones column for the softmax denominator), head-0 scores/exp
  with ctx trailing; remaining heads stream afterwards:
  S^T[sk,sq] = kT_h (stationary) x qT_h; exp via ACT with fused scale +
  per-partition mask bias; ctx^T and the denominator accumulate in PSUM over
  sk; normalize via reciprocal + ones-broadcast matmul; out = ctxT^T @ Wo_g.
"""

import numpy as np

B, SQ, SK, IN = 2, 1024, 4096, 1024
H_TOT, D, HPC = 16, 64, 4
DH = HPC * D  # 256, per-core head-dim slice
NCORES = 8

_CACHE = {}

# scheduling knobs (tuned against the cost-model timeline simulator)
CFG = {"pkv": 2, "pscore": 2, "sin": 3, "sexp": 10, "warm": 14,
       "kt1mode": 4, "osplit": 0, "csmall": 0, "ogq": 0, "warm2": 0,
       "warm3": 0, "lag": 2, "lag0": 4, "omerge": 1, "wkmid": 0,
       "o3tail": 0, "obufs": 6, "oalt": 0}


def _build(skp):
    import concourse.tile as tile
    from concourse import bacc, mybir

    FP = mybir.dt.float32
    FR = mybir.dt.float32r
    BF = mybir.dt.bfloat16
    AF = mybir.ActivationFunctionType
    MUL = mybir.AluOpType.mult
    ADD = mybir.AluOpType.add

    nc = bacc.Bacc("TRN2", target_bir_lowering=False, debug=False)

    qx_d = nc.dram_tensor("qx", [IN, SQ], BF, kind="ExternalInput").ap()
    kx_d = nc.dram_tensor("kx", [IN, skp], BF, kind="ExternalInput").ap()
    vx_d = nc.dram_tensor("vx", [IN, skp], BF, kind="ExternalInput").ap()
    wq_d = nc.dram_tensor("wq", [IN, DH], BF, kind="ExternalInput").ap()
    wk_d = nc.dram_tensor("wk", [IN, DH], BF, kind="ExternalInput").ap()
    wv_d = nc.dram_tensor("wv", [IN, DH], BF, kind="ExternalInput").ap()
    wo_d = nc.dram_tensor("wo", [DH, SQ], FR, kind="ExternalInput").ap()
    bqk_d = nc.dram_tensor("bqk", [128, 4], FP, kind="ExternalInput").ap()
    bvrep_d = nc.dram_tensor("bvrep", [128, DH], FP, kind="ExternalInput").ap()
    mb_d = nc.dram_tensor("mb", [128, skp // 128], FP, kind="ExternalInput").ap()
    ones_d = nc.dram_tensor("ones", [1, 128], FR, kind="ExternalInput").ap()
    out_d = nc.dram_tensor("out", [SQ, SQ], BF, kind="ExternalOutput").ap()

    NSKT = skp // 128          # sk tiles of 128
    NKC = IN // 128            # 8 contraction chunks
    SCALE = 1.0 / float(np.sqrt(D))

    with tile.TileContext(nc) as tc:
        # ---- resident tensors (one bufs=1 pool; distinct names = own slots) ----
        cpool_cm = tc.tile_pool(name="const", bufs=1)
        cpool = cpool_cm.__enter__()
        wq_sb = cpool.tile([128, NKC, DH], BF, name="wq_sb")
        wk_sb = cpool.tile([128, NKC, DH], BF, name="wk_sb")
        wv_sb = cpool.tile([128, NKC, DH], BF, name="wv_sb")
        wo_sb = cpool.tile([128, 2, SQ], FR, name="wo_sb")
        bqk_sb = cpool.tile([128, 4], FP, name="bqk_sb")
        bvrep_sb = cpool.tile([128, DH], FP, name="bvrep_sb")
        mb_sb = cpool.tile([128, NSKT], FP, name="mb_sb")
        ones_sb = cpool.tile([1, 128], FR, name="ones_sb")
        warm_sb = cpool.tile([1, 256], BF, name="warm_sb")
        qT_sb = cpool.tile([128, 2, SQ], FR, name="qT_sb")
        kT_sb = cpool.tile([128, 2, skp], FR, name="kT_sb")
        vext_sb = cpool.tile([128, NSKT, 65 * HPC], FR, name="vext_sb")
        ctxT_sb = cpool.tile([128, 2, SQ], FR, name="ctxT_sb")
        oA_sb = (cpool.tile([128, 8, SQ], mybir.dt.bfloat16, name="oA_sb")
                 if CFG.get("osplit", 0) else None)
        kx_sb = cpool.tile([128, NKC, skp], BF, name="kx_sb")

        # DMA priority order: Q-proj deps first, then K/V stream; wo is
        # loaded later (only needed for the output projection).
        nc.sync.dma_start(out=bqk_sb[:], in_=bqk_d[:, :])
        nc.sync.dma_start(out=wq_sb[:], in_=wq_d.rearrange("(kc p) n -> p kc n", p=128))

        with tc.tile_pool(name="sin", bufs=CFG["sin"]) as sin, \
             tc.tile_pool(name="sexp", bufs=CFG["sexp"]) as sexp, \
             tc.tile_pool(name="sout", bufs=CFG.get("sout", 2)) as sout, \
             tc.tile_pool(name="sspill", bufs=CFG.get("sspill", 2)) as sspill, \
             tc.tile_pool(name="pkv", bufs=CFG["pkv"], space="PSUM") as pkv, \
             tc.tile_pool(name="pscore", bufs=CFG["pscore"], space="PSUM") as pscore, \
             tc.tile_pool(name="pa", bufs=CFG.get("pa", 1), space="PSUM") as pa:

            # ---- PE warm-up: junk matmuls spend the p-state ramp during the
            # initial DMA wait ----
            nc.vector.memset(warm_sb[:], 0.0)

            def warm_fill(n):
                """Junk matmuls: keep the PE p-state ramp hot across known
                DMA-wait bubbles (idle resets the ramp to 2-3.7x slower)."""
                warm_ps = pkv.tile([1, 256], FP, tag="mm", name="warm_ps")
                for _ in range(n):
                    nc.tensor.matmul(warm_ps[0:1, 0:256],
                                     lhsT=warm_sb[0:1, 0:1],
                                     rhs=warm_sb[0:1, 0:256],
                                     start=True, stop=True)

            warm_fill(CFG["warm"])

            # ---- projections ----
            def blocks_of(width, blk=512):
                out, off = [], 0
                while off < width:
                    w = min(blk, width - off)
                    out.append((off, w))
                    off += w
                return out

            def load_x(x_d, off, w, eng, name="xin", dst=None):
                if dst is None:
                    xin = sin.tile([128, NKC, 512], BF, tag="sin", name=name)
                    ds = slice(0, w)
                else:
                    xin = dst
                    ds = slice(off, off + w)
                xr = x_d.rearrange("(kc p) n -> p kc n", p=128)
                h = w // 2 if w >= 512 else w
                eng.dma_start(out=xin[:, :, ds.start:ds.start + h],
                              in_=xr[:, :, off:off + h])
                if h < w:
                    eng.dma_start(out=xin[:, :, ds.start + h:ds.stop],
                                  in_=xr[:, :, off + h:off + w])
                return xin

            def proj_qk(w_sb, x_d, dst_sb, bias_col0, off, w, eng=None, xin=None,
                        ts=(0, 1), xoff=0):
                """out^T = W^T @ x^T (weight-stationary), bias-add -> fp32r."""
                if xin is None:
                    xin = load_x(x_d, off, w, eng or nc.sync)
                for t in ts:
                    ps = pkv.tile([128, 512], FP, tag="mm", name="ps")
                    for kc in range(NKC):
                        nc.tensor.matmul(
                            ps[:, 0:w],
                            lhsT=w_sb[:, kc, t * 128:(t + 1) * 128],
                            rhs=xin[:, kc, xoff:xoff + w],
                            start=(kc == 0), stop=(kc == NKC - 1))
                    with nc.allow_low_precision(reason="float32r storage"):
                        nc.vector.tensor_scalar_add(
                            dst_sb[:, t, off:off + w], ps[:, 0:w],
                            bqk_sb[:, bias_col0 + t:bias_col0 + t + 1])

            vv = vext_sb[:, :, :].rearrange("p s (h c) -> p s h c", c=65)

            def proj_v(xin, off, w):
                """v[sk, dh] directly: lhsT = value^T chunk, rhs = Wv."""
                for skt in range(off // 128, (off + w) // 128):
                    ps_v = pkv.tile([128, 256], FP, tag="mm", name="ps_v")
                    for kc in range(NKC):
                        nc.tensor.matmul(
                            ps_v[:, :],
                            lhsT=xin[:, kc, (skt * 128 - off):(skt * 128 - off) + 128],
                            rhs=wv_sb[:, kc, :],
                            start=(kc == 0), stop=(kc == NKC - 1))
                    with nc.allow_low_precision(reason="float32r storage"):
                        nc.vector.tensor_tensor(
                            vv[:, skt, :, 0:64],
                            ps_v[:, :].rearrange("p (h c) -> p h c", c=64),
                            bvrep_sb[:, :].rearrange("p (h c) -> p h c", c=64),
                            ADD)

            def scores_exp(h, skt):
                t, r0 = h // 2, 64 * (h % 2)
                ps_s = pscore.tile([128, SQ], FP, tag="mm", name="ps_s")
                for lo in range(0, SQ, 512):
                    nc.tensor.matmul(
                        ps_s[:, lo:lo + 512],
                        lhsT=kT_sb[r0:r0 + 64, t, skt * 128:(skt + 1) * 128],
                        rhs=qT_sb[r0:r0 + 64, t, lo:lo + 512],
                        start=True, stop=True)
                es = sexp.tile([128, SQ], FR, tag="es", name="es")
                nc.scalar.activation(
                    es[:, :], ps_s[:, :], AF.Exp,
                    bias=mb_sb[:, skt:skt + 1], scale=SCALE)
                return es

            def ctx_part(h, acc, skt, es, first, last):
                for lo in range(0, SQ, 512):
                    nc.tensor.matmul(
                        acc[:, lo:lo + 512],
                        lhsT=vext_sb[:, skt, 65 * h:65 * h + 65],
                        rhs=es[:, lo:lo + 512],
                        start=first, stop=last)

            def attn_skt(h, acc, skt, first=None, last=None):
                if first is None:
                    first = (skt == 0)
                if last is None:
                    last = (skt == NSKT - 1)
                es = scores_exp(h, skt)
                ctx_part(h, acc, skt, es, first, last)

            def spill_acc(acc):
                """Copy acc to SBUF so the psum accumulator frees immediately;
                the normalization chain then runs off the critical path."""
                spill = sspill.tile([65, SQ], FR, tag="spill", name="spill")
                with nc.allow_low_precision(reason="float32r storage"):
                    nc.vector.tensor_copy(spill[:], acc[:])
                return spill

            def normalize(h, spill, step=512, fast=False):
                """Chunked so downstream consumers unblock early; fast=True
                pipelines the chain across DVE/ACT for the last head, where
                the output projection is waiting on ctxT."""
                t, r0 = h // 2, 64 * (h % 2)
                rec = sout.tile([1, SQ], FR, tag="rec", name="rec")
                bc_sb = sout.tile([64, SQ], FP, tag="bc", name="bc_sb")
                for lo in range(0, SQ, step):
                    s = slice(lo, lo + step)
                    with nc.allow_low_precision(reason="float32r storage"):
                        nc.vector.reciprocal(rec[:, s], spill[64:65, s])
                    ps_bc = pkv.tile([64, 512], FP, tag="mm", name="ps_bc")
                    nc.tensor.matmul(ps_bc[:, 0:step],
                                     lhsT=ones_sb[0:1, 0:64],
                                     rhs=rec[0:1, s],
                                     start=True, stop=True)
                    if fast:
                        nc.scalar.copy(bc_sb[:, s], ps_bc[:, 0:step])
                    else:
                        nc.vector.tensor_copy(bc_sb[:, s], ps_bc[:, 0:step])
                    with nc.allow_low_precision(reason="float32r storage"):
                        nc.vector.tensor_tensor(ctxT_sb[r0:r0 + 64, t, s],
                                                spill[0:64, s], bc_sb[:, s], MUL)

            # vext ones columns (written once, before any v data lands)
            ones_fp = sout.tile([128, NSKT], FP, tag="onesfp", name="ones_fp")
            nc.vector.memset(ones_fp[:], 1.0)
            with nc.allow_low_precision(reason="float32r has float32 storage"):
                nc.vector.tensor_copy(
                    vv[:, :, :, 64:65],
                    ones_fp[:, :, None, None].to_broadcast((128, NSKT, HPC, 1)))

            # pass 1: Q proj, then per sk-block K/V proj, interleaved with
            # head-0 attention to keep ACT busy early
            for qbi, (off, w) in enumerate(blocks_of(SQ, CFG.get("qblk", 256))):
                if qbi == 2 and CFG.get("wkmid", 1):
                    nc.sync.dma_start(
                        out=wk_sb[:],
                        in_=wk_d.rearrange("(kc p) n -> p kc n", p=128))
                proj_qk(wq_sb, qx_d, qT_sb, 0, off, w, eng=nc.sync)
            nc.scalar.dma_start(out=bvrep_sb[:], in_=bvrep_d[:, :])
            nc.scalar.dma_start(out=mb_sb[:], in_=mb_d[:, :])
            nc.scalar.dma_start(out=ones_sb[:], in_=ones_d[:, :])
            acc = pa.tile([65, SQ], FP, tag="acc", name="acc")
            kblocks = blocks_of(skp)
            kt1 = CFG.get("kt1", 1)    # defer K-proj t=1 into head 1's sweep
            if len(blocks_of(skp)) > 5:
                kt1 = 0    # injection table only covers 5 blocks
            warm_fill(CFG.get("warm2", 0))
            corder = (kblocks[-1:] + kblocks[:-1]) if CFG.get("csmall", 1) \
                else list(kblocks)
            xk = load_x(kx_d, *corder[0], nc.sync, name="xk", dst=kx_sb)
            if not CFG.get("wkmid", 1):
                nc.sync.dma_start(
                    out=wk_sb[:],
                    in_=wk_d.rearrange("(kc p) n -> p kc n", p=128))
            nc.sync.dma_start(
                out=wv_sb[:], in_=wv_d.rearrange("(kc p) n -> p kc n", p=128))
            xv = load_x(vx_d, *corder[0], nc.sync, name="xv")
            ndone = 0
            pipe0 = []
            for bi, (off, w) in enumerate(corder):
                xk_cur, xv_cur = xk, xv
                if bi + 1 < len(corder):
                    xk = load_x(kx_d, *corder[bi + 1], nc.sync, name="xk",
                                dst=kx_sb)
                    xv = load_x(vx_d, *corder[bi + 1], nc.sync, name="xv")
                proj_qk(wk_sb, kx_d, kT_sb, 2, off, w, xin=kx_sb,
                        ts=(0,) if kt1 else (0, 1), xoff=off)
                if bi == 0:
                    warm_fill(CFG.get("warm3", 0))
                proj_v(xv_cur, off, w)
                for skt in range(off // 128, (off + w) // 128):
                    es = scores_exp(0, skt)
                    pipe0.append((skt, es))
                    if len(pipe0) > CFG.get("lag0", 2):
                        pskt, pes = pipe0.pop(0)
                        ctx_part(0, acc, pskt, pes, ndone == 0, False)
                        ndone += 1
            for pskt, pes in pipe0:
                ctx_part(0, acc, pskt, pes, ndone == 0, pskt == NSKT - 1)
                ndone += 1
            nc.scalar.dma_start(
                out=wo_sb[:], in_=wo_d.rearrange("(t p) n -> p t n", p=128))
            pending = (0, spill_acc(acc))

            # pass 2: remaining heads; head 1 (still on kT t=0) absorbs the
            # deferred K-proj t=1 matmuls to fill its exp-bound bubbles.
            # normalize(h-1) is emitted mid-sweep of head h so its PE/DVE work
            # lands in exp-bound bubbles instead of blocking the transition.
            def opartial(sq, half):
                """out-proj t=0 contribution for one 512-wide chunk, computed
                during pass-2 bubbles; gpsimd copies psum->sbuf (bf16)."""
                lo = half * 512
                ps = pkv.tile([128, 512], FP, tag="mm", name="ps_oa")
                nc.tensor.matmul(ps[:, :],
                                 lhsT=ctxT_sb[:, 0, sq * 128:(sq + 1) * 128],
                                 rhs=wo_sb[:, 0, lo:lo + 512],
                                 start=True, stop=True)
                with nc.allow_low_precision(reason="bf16 partial"):
                    nc.gpsimd.tensor_copy(oA_sb[:, sq, lo:lo + 512], ps[:, :])

            # injection schedule: {head: {skt: block_index}}
            OPART = {}
            _chunks = [(sq, half) for sq in range(8) for half in range(2)]
            for i, ch in enumerate(_chunks[:10]):
                OPART.setdefault((2, 7 + i), []).append(ch)
            for i, ch in enumerate(_chunks[10:]):
                OPART.setdefault((3, 1 + i), []).append(ch)
            km = CFG.get("kt1mode", 1)
            if km == 2:
                inj = {1: {0: 0, 6: 1, 12: 2}, 2: {0: 3, 4: 4}}
            elif km == 3:
                inj = {1: {0: 0, 8: 1}, 2: {0: 2, 4: 3, 8: 4}}
            elif km == 4:
                inj = {1: {0: 0, 4: 1, 8: 2}, 2: {0: 3, 8: 4}}
            else:
                inj = {1: {4 * b: b for b in range(len(kblocks))}}
            for h in range(1, HPC):
                acc = pa.tile([65, SQ], FP, tag="acc", name="acc")
                pipe = []
                for skt in range(NSKT):
                    bi = inj.get(h, {}).get(skt) if kt1 else None
                    if bi is not None and bi < len(kblocks):
                        boff, bw = kblocks[bi]
                        proj_qk(wk_sb, kx_d, kT_sb, 2, boff, bw,
                                xin=kx_sb, ts=(1,), xoff=boff)
                    if CFG.get("osplit", 0):
                        for sq, half in OPART.get((h, skt), ()):
                            opartial(sq, half)
                    es = scores_exp(h, skt)
                    pipe.append((skt, es))
                    if len(pipe) > CFG.get("lag", 2):
                        pskt, pes = pipe.pop(0)
                        ctx_part(h, acc, pskt, pes, pskt == 0, False)
                    if skt == 2 and pending is not None:
                        normalize(*pending)
                        pending = None
                for pskt, pes in pipe:
                    ctx_part(h, acc, pskt, pes, pskt == 0, pskt == NSKT - 1)
                if h < HPC - 1:
                    pending = (h, spill_acc(acc))
            # last head: normalize straight from psum (nothing waits on the
            # accumulator slot afterwards) so outproj can start immediately
            normalize(HPC - 1, acc, step=CFG.get("nstep", 256), fast=True)

            # ---- output projection: out = ctx @ Wo_g ----
            for sq in range(SQ // 128):
                ts_o = (1,) if CFG.get("osplit", 0) else (0, 1)
                if CFG.get("oalt", 1) and sq % 2 == 1:
                    # odd tiles accumulate in two pkv half-banks so the ps_o
                    # rings of consecutive tiles come from different pools
                    # (twice the pipeline depth against copy latency)
                    halves = [pkv.tile([128, 512], FP, tag="mm", name="ps_oh")
                              for _ in range(2)]
                    ps_o = None
                    for t in ts_o:
                        for hi, lo in enumerate(range(0, SQ, 512)):
                            nc.tensor.matmul(
                                halves[hi][:, :],
                                lhsT=ctxT_sb[:, t, sq * 128:(sq + 1) * 128],
                                rhs=wo_sb[:, t, lo:lo + 512],
                                start=(t == ts_o[0]), stop=(t == 1))
                else:
                    ps_o = pscore.tile([128, SQ], FP, tag="mm", name="ps_o")
                    halves = [ps_o[:, 0:512], ps_o[:, 512:1024]]
                    for t in ts_o:
                        for lo in range(0, SQ, 512):
                            nc.tensor.matmul(
                                ps_o[:, lo:lo + 512],
                                lhsT=ctxT_sb[:, t, sq * 128:(sq + 1) * 128],
                                rhs=wo_sb[:, t, lo:lo + 512],
                                start=(t == ts_o[0]), stop=(t == 1))
                o_sb = sout.tile([128, SQ], BF, tag="o", name="o_sb",
                                 bufs=CFG.get("obufs", 4))
                with nc.allow_low_precision(reason="bf16 output partials"):
                    nc.vector.tensor_copy(o_sb[:, 0:512], halves[0][:, :]
                                          if ps_o is None else ps_o[:, 0:512])
                    nc.scalar.copy(o_sb[:, 512:1024], halves[1][:, :]
                                   if ps_o is None else ps_o[:, 512:1024])
                if CFG.get("omerge", 0):
                    nc.sync.dma_start(
                        out=out_d[sq * 128:(sq + 1) * 128, :], in_=o_sb[:, :])
                else:
                    nc.sync.dma_start(
                        out=out_d[sq * 128:(sq + 1) * 128, 0:512],
                        in_=o_sb[:, 0:512])
                    nc.sync.dma_start(
                        out=out_d[sq * 128:(sq + 1) * 128, 512:1024],
                        in_=o_sb[:, 512:1024])

        cpool_cm.__exit__(None, None, None)

    nc.compile()
    return nc


def get_nc(skp=SK):
    key = ("nc", skp)
    if key not in _CACHE:
        _CACHE[key] = _build(skp)
    return _CACHE[key]


def make_in_maps(query, key, value, key_mask, Wq, bq, Wk, bk, Wv, bv, Wo, bo):
    import ml_dtypes
    BF = ml_dtypes.bfloat16
    f32 = lambda x: np.asarray(x, dtype=np.float32)
    query, key, value = f32(query), f32(key), f32(value)
    Wq, bq, Wk, bk = f32(Wq), f32(bq), f32(Wk), f32(bk)
    Wv, bv, Wo = f32(Wv), f32(bv), f32(Wo)
    key_mask = np.asarray(key_mask)

    # compact unmasked keys; pad to a common multiple of 128
    keep = [np.nonzero(key_mask[b] != 0)[0] for b in range(B)]
    skp = max(512, int(-(-max(len(k) for k in keep) // 128) * 128))
    skp = min(skp, SK)

    ones = np.ones((1, 128), np.float32)
    qx, kx, vx, mb = [], [], [], []
    for b in range(B):
        n = len(keep[b])
        kc = np.zeros((skp, IN), np.float32)
        vc = np.zeros((skp, IN), np.float32)
        kc[:n] = key[b][keep[b]]
        vc[:n] = value[b][keep[b]]
        mbias = np.full(skp, -1e9, np.float32)
        mbias[:n] = 0.0
        qx.append(np.ascontiguousarray(query[b].T.astype(BF)))
        kx.append(np.ascontiguousarray(kc.T.astype(BF)))
        vx.append(np.ascontiguousarray(vc.T.astype(BF)))
        mb.append(np.ascontiguousarray(mbias.reshape(skp // 128, 128).T))

    in_maps = []
    for c in range(NCORES):
        b, g = c // 4, c % 4
        S = slice(DH * g, DH * (g + 1))
        bqk = np.stack([bq[S][0:128], bq[S][128:256],
                        bk[S][0:128], bk[S][128:256]], axis=1)
        bvrep = np.broadcast_to(bv[S][None, :], (128, DH))
        in_maps.append({
            "qx": qx[b], "kx": kx[b], "vx": vx[b],
            "wq": np.ascontiguousarray(Wq[:, S].astype(BF)),
            "wk": np.ascontiguousarray(Wk[:, S].astype(BF)),
            "wv": np.ascontiguousarray(Wv[:, S].astype(BF)),
            "wo": np.ascontiguousarray(Wo[S, :]),
            "bqk": np.ascontiguousarray(bqk),
            "bvrep": np.ascontiguousarray(bvrep),
            "mb": mb[b], "ones": ones,
        })
    return in_maps, skp


def run(in_maps, skp=SK, trace=False):
    from concourse.bass_utils import run_bass_kernel_spmd
    nc = get_nc(skp)
    res = run_bass_kernel_spmd(nc, in_maps, list(range(NCORES)), trace=trace)
    _CACHE["last_results"] = res
    return res


def kernel(query, key, value, key_mask, Wq, bq, Wk, bk, Wv, bv, Wo, bo):
    in_maps, skp = make_in_maps(query, key, value, key_mask,
                                Wq, bq, Wk, bk, Wv, bv, Wo, bo)
    res = run(in_maps, skp)
    out = np.zeros((B, SQ, SQ), np.float32)
    for c in range(NCORES):
        out[c // 4] += np.asarray(res.results[c]["out"], np.float32)
    out += np.asarray(bo, np.float32)[None, None, :]
    return out
